# revision 36
# baseline (speedup 1.0000x reference)
"""BEV feature extractor (bilinear gather) on 8 Trainium2 NeuronCores.

Hardcoded problem: bev_feature [4,180,180,512] f32, batch_centers [4,2500,2]
f32, num_point=5 -> out [4,500,2560] f32.

Sharding: data-parallel over batch, 2 cores per batch splitting the 500
output rows into halves of 250. Each core bilinearly samples 1250 points
from its batch's map via SWDGE dma_gather over a host-built bf16
row-pairs tensor pairs[y,x] = (fmap[y,x], fmap[y+1,x]): ONE 4KB
descriptor per point fetches the whole 2x2 bilinear block (A=(y0,x0),
B=(y0+1,x0), C=(y0,x0+1), D=(y0+1,x0+1)). The pairs tensor is DECLARED
f32 on device (16-bit dtypes double the SWDGE descriptor-gen cost; the
gather is a byte mover, so SBUF slices are bitcast back to bf16). The 4
bilinear weights are applied per chunk as 2 muls on ACT + 2 fused
mul-adds + 1 add on DVE; results store as bf16 (host upcasts). bf16
keeps the end-to-end relative error at ~3.4e-3 against the f32
reference, well inside the 2e-2 gate, while halving gather bytes, store
bytes, and vector time.

Schedule notes (from perfetto traces): gathers alternate between SWDGE
queues 1/2 so back-to-back chunk transfers overlap; stores for chunks
0-7 ride SWDGE queue 0 (round-robins descriptors evenly over all 16 DMA
rings, where HWDGE glues ~55% of bytes to rings 0/1) while the last two
stores go HWDGE (sync/scalar) so GpSimd's final instruction retires
early and its ~6us end-of-kernel drain hides under the store tail.

Host work is input marshalling: the f32 grid-coordinate affine
((c+54)/0.075/8, matching the CPU reference's correctly-rounded
divisions), the point->slot permutation, floor/clip index +
bilinear-weight tables, and the bf16 row-pairs duplication of the
feature map.
"""

import os

import ml_dtypes
import numpy as np

BF16 = ml_dtypes.bfloat16

H = W = 180
C = 512
B = 4
NPT = 2500
NUM_POINT = 5
SEC = 500          # points per channel-block
ROWS = H * W       # 32400 flat pixel rows
NCHUNK = 10        # device chunks of 128 point-slots
PADN = NCHUNK * 128

_CACHE = {}
last_results = None  # BassKernelResults of the most recent run (for test.py)


def _build():
    import concourse.bacc as bacc
    import concourse.bass as bass
    import concourse.mybir as mybir
    import concourse.tile as tile
    from concourse.library_config import mlp

    f32 = mybir.dt.float32
    f16 = mybir.dt.bfloat16
    i16 = mybir.dt.int16
    Alu = mybir.AluOpType

    nc = bacc.Bacc("TRN2", target_bir_lowering=False, debug=False, num_swdge_queues=3)
    # row-pairs map: pairs[y*180+x] = [fmap[y,x], fmap[y+1,x]], fp16 bytes
    # DECLARED f32 (the gather is a byte mover; 16-bit dtypes double the
    # SWDGE descriptor-gen cost, so we gather "f32" and bitcast in SBUF)
    pairs = nc.dram_tensor("pairs", [ROWS, C], f32, kind="ExternalInput")
    # ACT scale APs must be f32: cols k=WAA, NCHUNK+k=WAB
    wts32 = nc.dram_tensor("wts32", [128, 2 * NCHUNK], f32, kind="ExternalInput")
    # DVE scalars, bf16: cols k=WBA, NCHUNK+k=WBB
    wts16 = nc.dram_tensor("wts16", [128, 2 * NCHUNK], f16, kind="ExternalInput")
    # 16-partition-wrapped gather indices, replicated x8 across partitions
    idxs = nc.dram_tensor("idxs", [128, 8 * NCHUNK], i16, kind="ExternalInput")
    out = nc.dram_tensor("out", [250, NUM_POINT, C], f16, kind="ExternalOutput")

    # overlapping view: elem at row r covers pair-rows r and r+1, i.e. the
    # full 2x2 pixel block [A|B|Cx|D] when r = y0*180+x0
    pview = bass.AP(pairs, 0, [[C, ROWS - 1], [1, 2 * C]])

    nc.gpsimd.load_library(mlp)
    with tile.TileContext(nc) as tc:
        with (
            tc.tile_pool(name="pc", bufs=1) as pc,
            tc.tile_pool(name="pa", bufs=10) as pa,
            tc.tile_pool(name="pt", bufs=12) as pt,
            tc.tile_pool(name="po", bufs=10) as po,
        ):
            IDX = pc.tile([128, 8 * NCHUNK], i16, tag="IDX")
            nc.sync.dma_start(IDX[:], idxs[:])
            W32 = pc.tile([128, 2 * NCHUNK], f32, tag="W32")
            nc.sync.dma_start(W32[:], wts32[:])
            W16 = pc.tile([128, 2 * NCHUNK], f16, tag="W16")
            nc.sync.dma_start(W16[:], wts16[:])

            # gathers on SWDGE queue 1 so the store descriptors (mainline
            # SWDGE queue 0) are not FIFO-ordered behind all gather traffic
            Gs = []
            for k in range(NCHUNK):
                G = pa.tile([128, 1, 2 * C], f32, tag="G")
                nc.gpsimd.dma_gather(
                    G[:], pview, IDX[:, 8 * k : 8 * (k + 1)],
                    128, 128, 2 * C, elem_step=C, queue_num=1 + (k % 2),
                )
                Gs.append(G)

            q = C // 2  # 1KB block = q f32 columns; bitcast to [128, C] fp16
            # ---- per-chunk weighted sum + store ----
            for k in range(NCHUNK):
                j, half = divmod(k, 2)
                cnt = 128 if half == 0 else 122
                G = Gs[k]
                sl = 0
                # fp16 block layout (f32 cols): A=[0:q] B=[q:2q] Cx=[2q:3q] D=[3q:4q]
                A16 = G[:, sl, 0:q].bitcast(f16)
                B16 = G[:, sl, q : 2 * q].bitcast(f16)
                C16 = G[:, sl, 2 * q : 3 * q].bitcast(f16)
                D16 = G[:, sl, 3 * q : 4 * q].bitcast(f16)
                # 2 muls on ACT, 2 fused mul-adds + 1 add on DVE
                t0 = pt.tile([128, C], f16, tag="t0")
                nc.scalar.mul(t0[:], A16, W32[:, k : k + 1])
                t1 = pt.tile([128, C], f16, tag="t1")
                nc.scalar.mul(t1[:], C16, W32[:, NCHUNK + k : NCHUNK + k + 1])
                s0 = pt.tile([128, C], f16, tag="s0")
                nc.vector.scalar_tensor_tensor(
                    s0[:], D16, W16[:, NCHUNK + k : NCHUNK + k + 1],
                    t0[:], Alu.mult, Alu.add,
                )
                s1 = pt.tile([128, C], f16, tag="s1")
                nc.vector.scalar_tensor_tensor(
                    s1[:], B16, W16[:, k : k + 1],
                    s0[:], Alu.mult, Alu.add,
                )
                o = po.tile([128, C], f16, tag="o")
                nc.vector.tensor_add(o[:], s1[:], t1[:])
                # SWDGE store for the early chunks (descriptors round-robin
                # evenly over rings); the last two go HWDGE so GpSimd's
                # final instruction retires early and its ~6us end-DRAIN
                # (Q7 handshake) hides under the store tail
                if k < NCHUNK - 2:
                    nc.gpsimd.dma_start(
                        out[half * 128 : half * 128 + cnt, j, :], o[:cnt, :]
                    )
                else:
                    eng = nc.sync if k % 2 == 0 else nc.scalar
                    eng.dma_start(
                        out[half * 128 : half * 128 + cnt, j, :], o[:cnt, :]
                    )

    nc.compile()
    return nc


def _prep_point_tables(cb, h):
    """cb [NPT, 2] f32 GRID coords for this batch; h in {0,1}.

    Computes in f32 (matching the reference's clip/floor semantics) the
    per-point gather indices and bilinear weights:
      xs = min(x, 179); x0 = floor(xs); fx = xs-x0; x1 = min(x0+1, 179);
      ax = x1-xs  (same for y); weights = outer products (cast fp16);
      idx rows use xb = min(x0, 178) so each gather elem covers the block.
    """
    f = np.float32
    pts = np.full((PADN, 2), f(90.0), dtype=np.float32)
    for k in range(NCHUNK):
        j, half = divmod(k, 2)
        cnt = 128 if half == 0 else 122
        p = np.arange(cnt)
        pts[k * 128 + p] = cb[j * SEC + h * 250 + half * 128 + p]

    xs = np.minimum(pts[:, 0], f(179.0))
    ys = np.minimum(pts[:, 1], f(179.0))
    x0 = np.floor(xs)
    y0 = np.floor(ys)
    fx = xs - x0
    fy = ys - y0
    x1 = np.minimum(x0 + f(1.0), f(179.0))
    y1 = np.minimum(y0 + f(1.0), f(179.0))
    ax = x1 - xs
    ay = y1 - ys
    waa = ax * ay
    wab = fx * ay
    wba = ax * fy
    wbb = fx * fy

    xb = np.minimum(x0, f(178.0)).astype(np.int32)
    ia = (y0.astype(np.int32) * W + xb).astype(np.int16)

    wts32 = np.empty((128, 2 * NCHUNK), np.float32)
    for arr, col0 in ((waa, 0), (wab, NCHUNK)):
        wts32[:, col0 : col0 + NCHUNK] = arr.reshape(NCHUNK, 128).T
    wts16 = np.empty((128, 2 * NCHUNK), BF16)
    for arr, col0 in ((wba, 0), (wbb, NCHUNK)):
        wts16[:, col0 : col0 + NCHUNK] = arr.reshape(NCHUNK, 128).T.astype(BF16)

    # dma_gather idx layout: [16, cols] wrapped, replicated x8. For point
    # slot p of chunk k the idx sits at [p%16, 8k + p//16].
    i = np.arange(PADN)
    k = i // 128
    p = i % 128
    idx16 = np.zeros((16, 8 * NCHUNK), np.int16)
    idx16[p % 16, 8 * k + p // 16] = ia
    idx = np.ascontiguousarray(np.tile(idx16, (8, 1)))
    return wts32, wts16, idx


def kernel(bev_feature, batch_centers, num_point=5):
    global last_results
    from concourse.bass_utils import run_bass_kernel_spmd

    assert int(num_point) == NUM_POINT
    bev = np.asarray(bev_feature, dtype=np.float32).reshape(B, ROWS, C)
    cen = np.asarray(batch_centers, dtype=np.float32)
    # grid coords, computed exactly like the f32 reference: (c+54)/0.075/8
    cen = (cen - np.float32(-54.0)) / np.float32(0.075) / np.float32(8.0)

    if "nc" not in _CACHE:
        _CACHE["nc"] = _build()
    nc = _CACHE["nc"]

    in_maps = []
    for b in range(B):
        # fp16 row-pairs duplication: pairs[r] = [fmap[r], fmap[r+180]],
        # viewed as f32 for the byte-moving gather (see _build)
        bev16 = bev[b].astype(BF16)
        P = np.empty((ROWS, 2, C), BF16)
        P[:, 0, :] = bev16
        P[: ROWS - W, 1, :] = bev16[W:]
        P[ROWS - W :, 1, :] = bev16[ROWS - W :]  # y=179: dup (weights are 0)
        P = P.reshape(ROWS, 2 * C).view(np.float32)
        for h in range(2):
            w32, w16, idx = _prep_point_tables(cen[b], h)
            in_maps.append({"pairs": P, "wts32": w32, "wts16": w16, "idxs": idx})

    trace = bool(os.environ.get("BEV_TRACE"))
    res = run_bass_kernel_spmd(nc, in_maps, list(range(8)), trace=trace)
    last_results = res

    full = np.empty((B, SEC, NUM_POINT * C), np.float32)
    for c in range(8):
        b, h = divmod(c, 2)
        full[b, h * 250 : (h + 1) * 250] = (
            res.results[c]["out"].astype(np.float32).reshape(250, NUM_POINT * C)
        )
    return full


# revision 37
# speedup vs baseline: 1.0229x; 1.0229x over previous
"""BEV feature extractor (bilinear gather) on 8 Trainium2 NeuronCores.

Hardcoded problem: bev_feature [4,180,180,512] f32, batch_centers [4,2500,2]
f32, num_point=5 -> out [4,500,2560] f32.

Sharding: data-parallel over batch, 2 cores per batch splitting the 500
output rows into halves of 250. Each core bilinearly samples 1250 points
from its batch's map via SWDGE dma_gather over a host-built bf16
row-pairs tensor pairs[y,x] = (fmap[y,x], fmap[y+1,x]): ONE 4KB
descriptor per point fetches the whole 2x2 bilinear block (A=(y0,x0),
B=(y0+1,x0), C=(y0,x0+1), D=(y0+1,x0+1)). The pairs tensor is DECLARED
f32 on device (16-bit dtypes double the SWDGE descriptor-gen cost; the
gather is a byte mover, so SBUF slices are bitcast back to bf16). The 4
bilinear weights are applied per chunk as 2 muls on ACT + 2 fused
mul-adds + 1 add on DVE; results store as bf16 (host upcasts). bf16
keeps the end-to-end relative error at ~3.4e-3 against the f32
reference, well inside the 2e-2 gate, while halving gather bytes, store
bytes, and vector time.

Schedule notes (from perfetto traces): gathers alternate between SWDGE
queues 1/2 so back-to-back chunk transfers overlap; stores for chunks
0-7 ride SWDGE queue 0 (round-robins descriptors evenly over all 16 DMA
rings, where HWDGE glues ~55% of bytes to rings 0/1) while the last two
stores go HWDGE (sync/scalar) so GpSimd's final instruction retires
early and its ~6us end-of-kernel drain hides under the store tail.

Host work is input marshalling: the f32 grid-coordinate affine
((c+54)/0.075/8, matching the CPU reference's correctly-rounded
divisions), the point->slot permutation, floor/clip index +
bilinear-weight tables, and the bf16 row-pairs duplication of the
feature map.
"""

import os

import ml_dtypes
import numpy as np

BF16 = ml_dtypes.bfloat16

H = W = 180
C = 512
B = 4
NPT = 2500
NUM_POINT = 5
SEC = 500          # points per channel-block
ROWS = H * W       # 32400 flat pixel rows
NCHUNK = 10        # device chunks of 128 point-slots
PADN = NCHUNK * 128

_CACHE = {}
last_results = None  # BassKernelResults of the most recent run (for test.py)


def _build():
    import concourse.bacc as bacc
    import concourse.bass as bass
    import concourse.mybir as mybir
    import concourse.tile as tile
    from concourse.library_config import mlp

    f32 = mybir.dt.float32
    f16 = mybir.dt.bfloat16
    i16 = mybir.dt.int16
    Alu = mybir.AluOpType

    nc = bacc.Bacc("TRN2", target_bir_lowering=False, debug=False, num_swdge_queues=3)
    # row-pairs map: pairs[y*180+x] = [fmap[y,x], fmap[y+1,x]], fp16 bytes
    # DECLARED f32 (the gather is a byte mover; 16-bit dtypes double the
    # SWDGE descriptor-gen cost, so we gather "f32" and bitcast in SBUF)
    pairs = nc.dram_tensor("pairs", [ROWS, C], f32, kind="ExternalInput")
    # ACT scale APs must be f32: cols k=WAA, NCHUNK+k=WAB
    wts32 = nc.dram_tensor("wts32", [128, 2 * NCHUNK], f32, kind="ExternalInput")
    # DVE scalars, bf16: cols k=WBA, NCHUNK+k=WBB
    wts16 = nc.dram_tensor("wts16", [128, 2 * NCHUNK], f16, kind="ExternalInput")
    # 16-partition-wrapped gather indices, replicated x8 across partitions
    idxs = nc.dram_tensor("idxs", [128, 8 * NCHUNK], i16, kind="ExternalInput")
    out = nc.dram_tensor("out", [250, NUM_POINT, C], f16, kind="ExternalOutput")

    # overlapping view: elem at row r covers pair-rows r and r+1, i.e. the
    # full 2x2 pixel block [A|B|Cx|D] when r = y0*180+x0
    pview = bass.AP(pairs, 0, [[C, ROWS - 1], [1, 2 * C]])

    nc.gpsimd.load_library(mlp)
    with tile.TileContext(nc) as tc:
        with (
            tc.tile_pool(name="pc", bufs=1) as pc,
            tc.tile_pool(name="pa", bufs=10) as pa,
            tc.tile_pool(name="pt", bufs=12) as pt,
            tc.tile_pool(name="po", bufs=10) as po,
        ):
            IDX = pc.tile([128, 8 * NCHUNK], i16, tag="IDX")
            nc.sync.dma_start(IDX[:], idxs[:])
            W32 = pc.tile([128, 2 * NCHUNK], f32, tag="W32")
            nc.sync.dma_start(W32[:], wts32[:])
            W16 = pc.tile([128, 2 * NCHUNK], f16, tag="W16")
            nc.sync.dma_start(W16[:], wts16[:])

            # gathers on SWDGE queue 1 so the store descriptors (mainline
            # SWDGE queue 0) are not FIFO-ordered behind all gather traffic
            Gs = []
            for k in range(NCHUNK):
                G = pa.tile([128, 1, 2 * C], f32, tag="G")
                nc.gpsimd.dma_gather(
                    G[:], pview, IDX[:, 8 * k : 8 * (k + 1)],
                    128, 128, 2 * C, elem_step=C, queue_num=1 + (k % 2),
                )
                Gs.append(G)

            q = C // 2  # 1KB block = q f32 columns; bitcast to [128, C] fp16
            # ---- per-chunk weighted sum + store ----
            for k in range(NCHUNK):
                j, half = divmod(k, 2)
                cnt = 128 if half == 0 else 122
                G = Gs[k]
                sl = 0
                # fp16 block layout (f32 cols): A=[0:q] B=[q:2q] Cx=[2q:3q] D=[3q:4q]
                A16 = G[:, sl, 0:q].bitcast(f16)
                B16 = G[:, sl, q : 2 * q].bitcast(f16)
                C16 = G[:, sl, 2 * q : 3 * q].bitcast(f16)
                D16 = G[:, sl, 3 * q : 4 * q].bitcast(f16)
                # 2 muls on ACT, 2 fused mul-adds + 1 add on DVE
                t0 = pt.tile([128, C], f16, tag="t0")
                nc.scalar.mul(t0[:], A16, W32[:, k : k + 1])
                t1 = pt.tile([128, C], f16, tag="t1")
                nc.scalar.mul(t1[:], C16, W32[:, NCHUNK + k : NCHUNK + k + 1])
                s0 = pt.tile([128, C], f16, tag="s0")
                nc.vector.scalar_tensor_tensor(
                    s0[:], D16, W16[:, NCHUNK + k : NCHUNK + k + 1],
                    t0[:], Alu.mult, Alu.add,
                )
                s1 = pt.tile([128, C], f16, tag="s1")
                nc.vector.scalar_tensor_tensor(
                    s1[:], B16, W16[:, k : k + 1],
                    s0[:], Alu.mult, Alu.add,
                )
                o = po.tile([128, C], f16, tag="o")
                nc.vector.tensor_add(o[:], s1[:], t1[:])
                # SWDGE store for the early chunks (descriptors round-robin
                # evenly over rings); the last two go HWDGE so GpSimd's
                # final instruction retires early and its ~6us end-DRAIN
                # (Q7 handshake) hides under the store tail
                if k < NCHUNK - 3:
                    nc.gpsimd.dma_start(
                        out[half * 128 : half * 128 + cnt, j, :], o[:cnt, :]
                    )
                else:
                    eng = nc.sync if k % 2 == 0 else nc.scalar
                    eng.dma_start(
                        out[half * 128 : half * 128 + cnt, j, :], o[:cnt, :]
                    )

    nc.compile()
    return nc


def _prep_point_tables(cb, h):
    """cb [NPT, 2] f32 GRID coords for this batch; h in {0,1}.

    Computes in f32 (matching the reference's clip/floor semantics) the
    per-point gather indices and bilinear weights:
      xs = min(x, 179); x0 = floor(xs); fx = xs-x0; x1 = min(x0+1, 179);
      ax = x1-xs  (same for y); weights = outer products (cast fp16);
      idx rows use xb = min(x0, 178) so each gather elem covers the block.
    """
    f = np.float32
    pts = np.full((PADN, 2), f(90.0), dtype=np.float32)
    for k in range(NCHUNK):
        j, half = divmod(k, 2)
        cnt = 128 if half == 0 else 122
        p = np.arange(cnt)
        pts[k * 128 + p] = cb[j * SEC + h * 250 + half * 128 + p]

    xs = np.minimum(pts[:, 0], f(179.0))
    ys = np.minimum(pts[:, 1], f(179.0))
    x0 = np.floor(xs)
    y0 = np.floor(ys)
    fx = xs - x0
    fy = ys - y0
    x1 = np.minimum(x0 + f(1.0), f(179.0))
    y1 = np.minimum(y0 + f(1.0), f(179.0))
    ax = x1 - xs
    ay = y1 - ys
    waa = ax * ay
    wab = fx * ay
    wba = ax * fy
    wbb = fx * fy

    xb = np.minimum(x0, f(178.0)).astype(np.int32)
    ia = (y0.astype(np.int32) * W + xb).astype(np.int16)

    wts32 = np.empty((128, 2 * NCHUNK), np.float32)
    for arr, col0 in ((waa, 0), (wab, NCHUNK)):
        wts32[:, col0 : col0 + NCHUNK] = arr.reshape(NCHUNK, 128).T
    wts16 = np.empty((128, 2 * NCHUNK), BF16)
    for arr, col0 in ((wba, 0), (wbb, NCHUNK)):
        wts16[:, col0 : col0 + NCHUNK] = arr.reshape(NCHUNK, 128).T.astype(BF16)

    # dma_gather idx layout: [16, cols] wrapped, replicated x8. For point
    # slot p of chunk k the idx sits at [p%16, 8k + p//16].
    i = np.arange(PADN)
    k = i // 128
    p = i % 128
    idx16 = np.zeros((16, 8 * NCHUNK), np.int16)
    idx16[p % 16, 8 * k + p // 16] = ia
    idx = np.ascontiguousarray(np.tile(idx16, (8, 1)))
    return wts32, wts16, idx


def kernel(bev_feature, batch_centers, num_point=5):
    global last_results
    from concourse.bass_utils import run_bass_kernel_spmd

    assert int(num_point) == NUM_POINT
    bev = np.asarray(bev_feature, dtype=np.float32).reshape(B, ROWS, C)
    cen = np.asarray(batch_centers, dtype=np.float32)
    # grid coords, computed exactly like the f32 reference: (c+54)/0.075/8
    cen = (cen - np.float32(-54.0)) / np.float32(0.075) / np.float32(8.0)

    if "nc" not in _CACHE:
        _CACHE["nc"] = _build()
    nc = _CACHE["nc"]

    in_maps = []
    for b in range(B):
        # fp16 row-pairs duplication: pairs[r] = [fmap[r], fmap[r+180]],
        # viewed as f32 for the byte-moving gather (see _build)
        bev16 = bev[b].astype(BF16)
        P = np.empty((ROWS, 2, C), BF16)
        P[:, 0, :] = bev16
        P[: ROWS - W, 1, :] = bev16[W:]
        P[ROWS - W :, 1, :] = bev16[ROWS - W :]  # y=179: dup (weights are 0)
        P = P.reshape(ROWS, 2 * C).view(np.float32)
        for h in range(2):
            w32, w16, idx = _prep_point_tables(cen[b], h)
            in_maps.append({"pairs": P, "wts32": w32, "wts16": w16, "idxs": idx})

    trace = bool(os.environ.get("BEV_TRACE"))
    res = run_bass_kernel_spmd(nc, in_maps, list(range(8)), trace=trace)
    last_results = res

    full = np.empty((B, SEC, NUM_POINT * C), np.float32)
    for c in range(8):
        b, h = divmod(c, 2)
        full[b, h * 250 : (h + 1) * 250] = (
            res.results[c]["out"].astype(np.float32).reshape(250, NUM_POINT * C)
        )
    return full
